# revision 26
# baseline (speedup 1.0000x reference)
"""Trainium2 Bass kernel for nn_CrossAttention (16x6209x256 cross-attention).

Strategy
--------
Data-parallel over batch: 16 batches -> 8 cores x 2 batches. Each core runs an
identical Bass/Tile program on its own batch slice (pure SPMD, no collectives).

Per batch the math is
    mapped_a = a @ Wa + ba            [6209, 64]
    mapped_b = b @ Wb + bb            [256, 64]
    scores   = mapped_a @ mapped_b.T * 8
    attn     = softmax(scores, -1)
    out      = (attn @ mapped_b) @ Wc + bc

With no nonlinearity between the projections and the attention matmuls, the
small weights fold per batch (computed on device in exact fp32):
    Wfused    = 8 * Wa @ mapped_b.T               [256, 256]
    scoreBias = 8 * ba @ mapped_b.T               [256]
    Wout      = mapped_b @ Wc + 1 x bc            [256, 256]
    scores    = a @ Wfused + scoreBias
    out       = softmax(scores) @ Wout        (bias bc exact since rows sum to 1)

Precision: softmax amplifies score error by |scores| (~900 here), so the
scores matmul runs as a 3-term bf16 split (a = ahi+alo split on host,
Wfused = Whi+Wlo split on device): scores ~ ahi@Whi + alo@Whi + ahi@Wlo,
residual ~1e-4 absolute. Weight prep runs in exact fp32.

Softmax normalization is moved to the HOST: the device ships the
unnormalized exp(scores - rowmax) matrix through the AV matmul plus the
per-row sumexp; the host divides. Since Wout rows carry bc, dividing the
unnormalized (exp @ Wout) by sumexp keeps the bc term exact. This removes
the on-device reciprocal+scale pass, lets attn/attnT/Wout/out all live in
fp16 (validated: rel err 8e-4 vs 2e-2 tolerance), halves the output DMA,
and makes the PE transposes run at 1 cycle/row.

Layout: input_a is transposed on host to [256, seq] (padded to 6272 = 49*128
so every subtile is full 128 rows) so the contraction dim arrives on SBUF
partitions straight from DMA; output is produced transposed fp16 [256, seq]
and divided/transposed back on host. sumexp is a DVE fp16 free-dim reduce
(GpSimd cannot reduce along the free dim) shipped as [128, 49] f32 per
batch. A burst of dummy eye-matmuls at program start keeps the PE busy while
the first input macro DMAs in, so the HAM clock-gate is already released
(2.4 GHz) when real work starts.
"""
import sys

for _p in ("/opt/trn_rl_repo",):
    if _p not in sys.path:
        sys.path.append(_p)

import numpy as np
import ml_dtypes

import concourse.bacc as bacc
import concourse.mybir as mybir
import concourse.tile as tile
from concourse.bass_utils import run_bass_kernel_spmd

F32 = mybir.dt.float32
F32R = mybir.dt.float32r
BF16 = mybir.dt.bfloat16
F16 = mybir.dt.float16
P = 128

N_CORES = 8
BATCHES_PER_CORE = 2
SEQ = 6209
SEQ_PAD = 6272   # 49 * 128
N_SUB = SEQ_PAD // P
DF = 256          # feature dim of a / b
HID = 64          # projection dim
DMA_MACRO = 2048  # rows fetched/stored per DMA instruction
CMACRO = 512      # rows per compute macro (4 subtiles of 128)
N_WARM = 40       # dummy matmuls to trip the HAM clock gate at t=0
# graded macro sizes: two small first so compute starts as soon as possible
# with two chunks in flight (the first transfers race the HAM warmup),
# small last for a short tail
MACROS = (512, 512, 2048, 2048, 1024, 128)
assert sum(MACROS) == SEQ_PAD


def _row_plan(n_rows):
    """[(dma_start, dma_len, [(cm_start_within_dma, cm_len), ...]), ...]"""
    plan = []
    pos = 0
    mi = 0
    while pos < n_rows:
        d = min(MACROS[mi] if mi < len(MACROS) else DMA_MACRO, n_rows - pos)
        cms = []
        q = 0
        while q < d:
            c = min(CMACRO, d - q)
            cms.append((q, c))
            q += c
        plan.append((pos, d, cms))
        pos += d
        mi += 1
    return plan


def build_program(seq=SEQ_PAD, batches=BATCHES_PER_CORE, use_ba=False):
    nc = bacc.Bacc("TRN2", target_bir_lowering=False, debug=False)

    a_hl = nc.dram_tensor("a_hl", [batches, 2 * DF, seq], BF16, kind="ExternalInput")
    b_t = nc.dram_tensor("b_t", [batches, DF, DF], F32, kind="ExternalInput")
    wat = nc.dram_tensor("wat", [HID, DF], F32, kind="ExternalInput")
    wb = nc.dram_tensor("wb", [DF, HID], F32, kind="ExternalInput")
    wc = nc.dram_tensor("wc", [HID, DF], F32, kind="ExternalInput")
    ba_d = nc.dram_tensor("ba_d", [HID, 1], F32, kind="ExternalInput")
    bb_d = nc.dram_tensor("bb_d", [HID, 1], F32, kind="ExternalInput")
    bc_d = nc.dram_tensor("bc_d", [1, DF], F32, kind="ExternalInput")
    eye_d = nc.dram_tensor("eye_d", [P, P], F16, kind="ExternalInput")
    ones_d = nc.dram_tensor("ones_d", [1, P], F32, kind="ExternalInput")
    out_t = nc.dram_tensor("out_t", [batches, DF, seq], F16, kind="ExternalOutput")
    sum_t = nc.dram_tensor("sum_t", [batches, P, P], F32,
                           kind="ExternalOutput")

    Exp = mybir.ActivationFunctionType.Exp
    Copy = mybir.ActivationFunctionType.Copy
    Ident = mybir.ActivationFunctionType.Identity

    with tile.TileContext(nc) as tc:
        with (
            tc.tile_pool(name="const", bufs=1) as cpool,
            tc.tile_pool(name="wpool", bufs=2) as wpool,
            tc.tile_pool(name="apool", bufs=3) as apool,
            tc.tile_pool(name="mpool", bufs=2) as mpool,
            tc.tile_pool(name="spool", bufs=2) as spool,
            tc.tile_pool(name="opool", bufs=3) as opool,
            tc.tile_pool(name="pp", bufs=1, space="PSUM") as pp,
        ):
            plan = _row_plan(seq)
            items = [(b, mi) for b in range(batches) for mi in range(len(plan))]
            aT_tiles = {}

            def issue_aT(b, mi):
                d0, dlen, _ = plan[mi]
                t = apool.tile([P, 4, DMA_MACRO], BF16, tag="aT")
                nc.sync.dma_start(
                    t[:, :, :dlen],
                    a_hl[b][:, d0:d0 + dlen].rearrange("(k p) i -> p k i", p=P),
                )
                aT_tiles[(b, mi)] = t

            # ---- head DMA order matters (one queue, in-order): eye first
            # (warmup), then b_t[0] (weight prep), then the small first input
            # macro, then the rest of the constants ----
            eye_sb = cpool.tile([P, P], F16)
            nc.sync.dma_start(eye_sb[:], eye_d[:])
            # b_t host pre-arranged to [p, k, j] so the DMA is contiguous
            # 2KB lines; the strided rearrange ran at ~65GB/s
            bT_sbs = []
            bT_sb0 = wpool.tile([P, 2, DF], F32)
            nc.sync.dma_start(bT_sb0[:], b_t[0].rearrange("(p k) j -> p k j", p=P))
            bT_sbs.append(bT_sb0)
            issue_aT(0, 0)
            issue_aT(0, 1)
            wat_sb = cpool.tile([HID, DF], F32)
            nc.sync.dma_start(wat_sb[:], wat[:])
            wb_sb = cpool.tile([P, 2, HID], F32)
            nc.sync.dma_start(wb_sb[:], wb[:].rearrange("(k p) h -> p k h", p=P))
            wc_sb = cpool.tile([HID, DF], F32)
            nc.sync.dma_start(wc_sb[:], wc[:])
            ba_sb = cpool.tile([HID, 1], F32)
            nc.sync.dma_start(ba_sb[:], ba_d[:])
            bb_sb = cpool.tile([HID, 1], F32)
            nc.sync.dma_start(bb_sb[:], bb_d[:])
            bc_sb = cpool.tile([1, DF], F32)
            nc.sync.dma_start(bc_sb[:], bc_d[:])
            ones_sb = cpool.tile([1, P], F32)
            nc.sync.dma_start(ones_sb[:], ones_d[:])
            for b in range(1, batches):
                bT_sb = wpool.tile([P, 2, DF], F32)
                nc.sync.dma_start(bT_sb[:], b_t[b].rearrange("(p k) j -> p k j", p=P))
                bT_sbs.append(bT_sb)

            # ---- HAM warmup: keep PE busy while the first macro DMAs in ----
            warm_ps = pp.tile([P, CMACRO], F32, tag="fin0")
            for w in range(N_WARM):
                nc.tensor.matmul(
                    warm_ps[:, :P], eye_sb[:], eye_sb[:],
                    start=True, stop=True,
                )

            # ---- fused weights for ALL batches upfront (exact fp32 matmuls);
            # runs while the first input macro is still streaming in, and
            # removes the PE-starving prep chain at the batch boundary ----
            whis, wlos, wos, sbiases = [], [], [], []
            for b in range(batches):
                bT_sb = bT_sbs[b]

                ps_mb = pp.tile([HID, DF], F32, tag="fin1")
                for k in range(2):
                    nc.tensor.matmul(
                        ps_mb[:],
                        wb_sb[:, k, :],
                        bT_sb[:, k, :],
                        start=(k == 0), stop=(k == 1),
                    )
                mapped_bT = wpool.tile([HID, DF], F32)
                nc.scalar.activation(mapped_bT[:], ps_mb[:], Ident, bias=bb_sb[:])

                # Wfused, split hi/lo into bf16 (scale 8 folded in)
                whi_sb = wpool.tile([P, 2, DF], BF16)
                wlo_sb = wpool.tile([P, 2, DF], BF16)
                for c in range(2):
                    ps_wf = pp.tile([P, DF], F32, tag="fin1")
                    nc.tensor.matmul(
                        ps_wf[:],
                        wat_sb[:, c * P:(c + 1) * P],
                        mapped_bT[:],
                        start=True, stop=True,
                    )
                    nc.scalar.activation(whi_sb[:, c, :], ps_wf[:], Copy, scale=8.0)
                    # wlo = 8*wf - whi (rounded to bf16)
                    nc.vector.scalar_tensor_tensor(
                        wlo_sb[:, c, :],
                        ps_wf[:],
                        8.0,
                        whi_sb[:, c, :],
                        op0=mybir.AluOpType.mult,
                        op1=mybir.AluOpType.subtract,
                    )

                if use_ba:
                    ps_sbias = pp.tile([1, DF], F32, tag="fin1")
                    nc.tensor.matmul(
                        ps_sbias[:],
                        ba_sb[:],
                        mapped_bT[:],
                        start=True, stop=True,
                    )
                    sbias_sb = wpool.tile([1, DF], F32)
                    nc.scalar.activation(sbias_sb[:], ps_sbias[:], Copy, scale=8.0)
                    sbiases.append(sbias_sb)

                # Wout in fp16 (attn is unnormalized fp16 exp; host divides)
                wo_sb = wpool.tile([P, 2, DF], F16)
                for c in range(2):
                    ps_wo = pp.tile([P, DF], F32, tag="fin1")
                    nc.tensor.matmul(
                        ps_wo[:],
                        mapped_bT[:, c * P:(c + 1) * P],
                        wc_sb[:],
                        start=True, stop=False,
                    )
                    nc.tensor.matmul(
                        ps_wo[:],
                        ones_sb[:],
                        bc_sb[:],
                        start=False, stop=True,
                    )
                    nc.vector.tensor_copy(wo_sb[:, c, :], ps_wo[:])
                whis.append(whi_sb)
                wlos.append(wlo_sb)
                wos.append(wo_sb)

            # second warmup burst: bridges the gap between prep and the
            # first chunk so the HAM clock gate never re-engages during
            # the pipeline fill
            for w in range(16):
                nc.tensor.matmul(
                    warm_ps[:, :P], eye_sb[:], eye_sb[:],
                    start=True, stop=True,
                )

            # ---- main loops, no prep between batches ----
            for b in range(batches):
                whi_sb, wlo_sb, wo_sb = whis[b], wlos[b], wos[b]
                if use_ba:
                    sbias_sb = sbiases[b]
                # one sumexp tile per batch, single DMA at batch end
                sum_sb = spool.tile([P, P], F32, tag="sum")
                for mi, (d0, dlen, cms) in enumerate(plan):
                    # prefetch two ahead (0 and 1 were issued in the head):
                    # the issue lands before this macro's output DMAs enter
                    # the (in-order) Sync queue
                    idx = b * len(plan) + mi
                    if idx + 2 < len(items):
                        issue_aT(*items[idx + 2])
                    aT_sb = aT_tiles.pop((b, mi))

                    for mo, R in cms:
                        ns = R // P
                        subs = [(o, P) for o in range(0, R, P)]

                        scores_ps = pp.tile([P, 4 * DF], F32, tag="scores", bufs=2)
                        for s, (io, r) in enumerate(subs):
                            c0 = s * DF
                            terms = []
                            for k in range(2):
                                ah = aT_sb[:, k, mo + io:mo + io + r]
                                al = aT_sb[:, 2 + k, mo + io:mo + io + r]
                                terms += [
                                    (ah, whi_sb[:, k, :]),
                                    (al, whi_sb[:, k, :]),
                                    (ah, wlo_sb[:, k, :]),
                                ]
                            for t, (lhs, rhs) in enumerate(terms):
                                nc.tensor.matmul(
                                    scores_ps[:r, c0:c0 + DF],
                                    lhs,
                                    rhs,
                                    start=(t == 0),
                                    stop=(t == len(terms) - 1) and not use_ba,
                                )
                            if use_ba:
                                nc.tensor.matmul(
                                    scores_ps[:r, c0:c0 + DF],
                                    ones_sb[:, :r],
                                    sbias_sb[:],
                                    start=False, stop=True,
                                )

                        negmax = mpool.tile([P, 4], F32, tag="negmax")
                        nc.vector.tensor_reduce(
                            negmax[:, :ns],
                            scores_ps[:, :ns * DF].rearrange(
                                "p (s j) -> p s j", s=ns),
                            axis=mybir.AxisListType.X,
                            op=mybir.AluOpType.max,
                            negate=True,
                        )

                        # unnormalized exp in fp16; sumexp on GpSimd
                        attn_sb = mpool.tile([P, 4, DF], F16, tag="attn")
                        for s, (io, r) in enumerate(subs):
                            nc.scalar.activation(
                                attn_sb[:, s, :],
                                scores_ps[:, s * DF:(s + 1) * DF],
                                Exp,
                                bias=negmax[:, s:s + 1],
                            )
                        # 2-stage sumexp: fp16 pairwise add (2x DVE rate),
                        # then f32 reduce over half the width
                        sumtmp = mpool.tile([P, 4, P], F16, tag="sumtmp")
                        nc.vector.tensor_tensor(
                            sumtmp[:, :ns, :],
                            attn_sb[:, :ns, 0:P],
                            attn_sb[:, :ns, P:2 * P],
                            op=mybir.AluOpType.add,
                        )
                        s0 = (d0 + mo) // P
                        nc.vector.tensor_reduce(
                            sum_sb[:, s0:s0 + ns],
                            sumtmp[:, :ns, :],
                            axis=mybir.AxisListType.X,
                            op=mybir.AluOpType.add,
                        )

                        aT0_ps = pp.tile([P, CMACRO], F16, tag="attnT0")
                        aT1_ps = pp.tile([P, CMACRO], F16, tag="attnT1")
                        for s, (io, r) in enumerate(subs):
                            for jh, dst in ((0, aT0_ps), (1, aT1_ps)):
                                nc.tensor.transpose(
                                    dst[:, io:io + r],
                                    attn_sb[:, s, jh * P:(jh + 1) * P],
                                    eye_sb[:],
                                )
                        attnT0 = mpool.tile([P, CMACRO], F16, tag="attnT0sb")
                        attnT1 = mpool.tile([P, CMACRO], F16, tag="attnT1sb")
                        nc.scalar.copy(attnT0[:, :R], aT0_ps[:, :R])
                        nc.vector.tensor_copy(attnT1[:, :R], aT1_ps[:, :R])

                        # final: outT[fo, i] = sum_j Wout[j, fo] attnT[j, i]
                        outT_sb = opool.tile([P, 2, CMACRO], F16, tag="outT")
                        for c in range(2):
                            ps_fin = pp.tile([P, CMACRO], F32, tag=f"fin{c}")
                            for k, aTk in enumerate((attnT0, attnT1)):
                                nc.tensor.matmul(
                                    ps_fin[:, :R],
                                    wo_sb[:, k, c * P:(c + 1) * P],
                                    aTk[:, :R],
                                    start=(k == 0), stop=(k == 1),
                                )
                            if c == 0:
                                nc.vector.tensor_copy(
                                    outT_sb[:, c, :R], ps_fin[:, :R])
                            else:
                                nc.scalar.copy(
                                    outT_sb[:, c, :R], ps_fin[:, :R])

                        # per-chunk output DMA (hardware queue; GpSimd-issued
                        # DMAs go through SWDGE which added ~8us of tail)
                        nc.sync.dma_start(
                            out_t[b][:, d0 + mo:d0 + mo + R].rearrange(
                                "(c p) i -> p c i", p=P),
                            outT_sb[:, :, :R],
                        )

                nc.sync.dma_start(sum_t[b][:], sum_sb[:])

    nc.compile()
    return nc


_PROGRAM_CACHE = {}


def _get_program(seq=SEQ_PAD, batches=BATCHES_PER_CORE, use_ba=False):
    key = (seq, batches, use_ba)
    if key not in _PROGRAM_CACHE:
        _PROGRAM_CACHE[key] = build_program(seq, batches, use_ba)
    return _PROGRAM_CACHE[key]


def make_in_maps(input_a, input_b, Wa, ba, Wb, bb, Wc, bc,
                 n_cores=N_CORES, batches=BATCHES_PER_CORE):
    input_a = np.asarray(input_a, dtype=np.float32)
    input_b = np.asarray(input_b, dtype=np.float32)
    nb = input_a.shape[0]
    a_t = np.zeros((nb, DF, SEQ_PAD), dtype=np.float32)
    a_t[:, :, :SEQ] = input_a.transpose(0, 2, 1)
    a_hi = a_t.astype(ml_dtypes.bfloat16)
    a_lo = (a_t - a_hi.astype(np.float32)).astype(ml_dtypes.bfloat16)
    # rows 0..DF-1 = hi, DF..2DF-1 = lo  -> [B, 2*DF, seq]
    a_hl = np.ascontiguousarray(np.concatenate([a_hi, a_lo], axis=1))
    # b^T with rows pre-permuted to [p, k] order so the device DMA into a
    # [128, 2, 256] tile reads contiguous 2KB lines per partition
    b_t = input_b.transpose(0, 2, 1).reshape(-1, 2, P, DF)
    b_t = np.ascontiguousarray(b_t.transpose(0, 2, 1, 3).reshape(-1, DF, DF))
    shared = {
        "wat": np.ascontiguousarray(np.asarray(Wa, np.float32).T),
        "wb": np.ascontiguousarray(np.asarray(Wb, np.float32)),
        "wc": np.ascontiguousarray(np.asarray(Wc, np.float32)),
        "ba_d": np.asarray(ba, np.float32).reshape(HID, 1).copy(),
        "bb_d": np.asarray(bb, np.float32).reshape(HID, 1).copy(),
        "bc_d": np.asarray(bc, np.float32).reshape(1, DF).copy(),
        "eye_d": np.eye(P, dtype=np.float16),
        "ones_d": np.ones((1, P), dtype=np.float32),
    }
    in_maps = []
    for c in range(n_cores):
        lo, hi = c * batches, (c + 1) * batches
        in_maps.append({
            "a_hl": np.ascontiguousarray(a_hl[lo:hi]),
            "b_t": np.ascontiguousarray(b_t[lo:hi]),
            **shared,
        })
    return in_maps


def assemble_output(results):
    """results: list of per-core dicts with out_t [nb, 256, SEQ_PAD] f16 and
    sum_t [nb, 128, N_SUB] f32 -> full [16, SEQ, 256] f32."""
    outs = np.concatenate([np.asarray(r["out_t"]) for r in results], axis=0)
    sums = np.concatenate([np.asarray(r["sum_t"])[:, :, :N_SUB] for r in results], axis=0)
    # sum_t col s, partition p -> seq index s*128+p
    se = sums.transpose(0, 2, 1).reshape(sums.shape[0], SEQ_PAD)
    out = outs[:, :, :SEQ].astype(np.float32) / se[:, None, :SEQ]
    return np.ascontiguousarray(out.transpose(0, 2, 1))


def kernel(input_a, input_b, Wa, ba, Wb, bb, Wc, bc):
    use_ba = bool(np.any(np.asarray(ba)))
    nc = _get_program(use_ba=use_ba)
    in_maps = make_in_maps(input_a, input_b, Wa, ba, Wb, bb, Wc, bc)
    res = run_bass_kernel_spmd(nc, in_maps, core_ids=list(range(N_CORES)))
    return assemble_output(res.results)
